# revision 5
# baseline (speedup 1.0000x reference)
"""Trainium2 Bass kernel for DiscreteTimeS4.

Reference computation (per batch element b):
    x_proj = relu(x @ Wi^T + bi)          [T, P]
    u      = x_proj @ B                   [T, H]
    h_t    = a * h_{t-1} + u_t            (diagonal linear scan over T)
    y      = hs @ C                       [T, P]
    out    = y @ Wo^T + bo                [T, O]

Sharding: data-parallel over the batch axis — core b handles x_seq[b].
Weights replicated. No cross-device communication.

Device strategy (per core, all matmuls in fp32r = full-rate PE):
  - host pre-transposes x to x^T [D, T] so the contraction dim (D) sits on
    SBUF partitions; host also fuses W2 = C @ Wo^T so stages 4+5 collapse
    into one matmul; all [128, *] weights are packed into ONE DRAM tensor so
    startup is a single large DMA.
  - pipeline over time chunks of 512 (PE software-pipelined: chunk c's last
    matmul stage runs while chunk c+1's first stages execute, so the PE
    never waits on the scan):
      MM1: XP^T[p,t] = Wi @ x^T          (lhsT = Wi^T, K=512)
      ACT: relu(psum + bi) -> SBUF (fp32r)
      MM2: U^T[h,t] = B^T @ XP^T         (lhsT = B, K=256)
      DVE: tensor_tensor_scan: h = a*h + u along t (fp32 state, carry
           chained across chunks via initial=prev[:, -1:], decay broadcast
           via a zero-stride AP)
      MM3: OUT[t,o] = (HS^T)^T @ W2 + bo (lhsT = HS^T tile -> natural [t,o]
           output layout; bo folded in as a K=1 matmul of ones^T @ bo_row
           for half the tiles, DVE tensor_add for the rest)
      copy psum -> SBUF (ACT / DVE split), DMA out on the GPSIMD (SWDGE)
      ring so stores never queue behind loads.
"""

import numpy as np

try:
    import concourse.bass as bass
except ImportError:  # pragma: no cover
    import sys

    sys.path.insert(0, "/opt/trn_rl_repo")
    import concourse.bass as bass

from contextlib import ExitStack

import concourse.mybir as mybir
import concourse.tile as tile
from concourse import bacc
from concourse.bass import ts
from concourse.bass_utils import run_bass_kernel_spmd

BSZ, T, D, P, H, O = 8, 4096, 512, 256, 256, 512
CH = 512  # time-chunk (free dim of MM1/MM2, PSUM bank = 512 fp32)
NCH = T // CH
F32 = mybir.dt.float32
F32R = mybir.dt.float32r

# per chunk: how many of the 4 output t-subtiles take the DVE bias-add path
# (remaining go through a K=1 bias matmul on PE + plain ACT copy)
N_DVE_BIAS = 2

KD = D // 128  # 4 k-tiles for MM1
KP = P // 128  # 2
KH = H // 128  # 2
MO = O // 128  # 4 out t-subtiles per chunk

# packed weight layout (free-dim offsets, in fp32 elements)
WI_OFF = 0  # [128, KD*P]
B_OFF = WI_OFF + KD * P  # [128, KP*H]
W2_OFF = B_OFF + KP * H  # [128, KH*O]
WPACK_F = W2_OFF + KH * O

_NC_CACHE = {}


def _bcast_free(ap, n):
    """Broadcast a [128, 1] AP along the free dim via zero stride."""
    return bass.AP(tensor=ap.tensor, offset=ap.offset, ap=[list(ap.ap[0]), [0, n]])


def build_nc(mm_dt=F32R, n_dve_bias=N_DVE_BIAS, nch=NCH):
    key = (mm_dt, n_dve_bias, nch)
    if key in _NC_CACHE:
        return _NC_CACHE[key]
    wdt = mm_dt
    nc = bacc.Bacc("TRN2", target_bir_lowering=False, debug=False)

    xT_d = nc.dram_tensor("xT", [D, T], wdt, kind="ExternalInput")
    wpack_d = nc.dram_tensor("wpack", [128, WPACK_F], wdt, kind="ExternalInput")
    mpack_d = nc.dram_tensor("mpack", [128, KP + KH], F32, kind="ExternalInput")
    ones_d = nc.dram_tensor("ones", [1, 128], wdt, kind="ExternalInput")
    borow_d = nc.dram_tensor("borow", [1, O], wdt, kind="ExternalInput")
    borowf_d = nc.dram_tensor("borowf", [1, O], F32, kind="ExternalInput")
    out_d = nc.dram_tensor("out", [T, O], F32, kind="ExternalOutput")

    with tile.TileContext(nc) as tc, ExitStack() as ctx:
        wpool = ctx.enter_context(tc.tile_pool(name="weights", bufs=1))
        xpool = ctx.enter_context(tc.tile_pool(name="x", bufs=NCH))
        xppool = ctx.enter_context(tc.tile_pool(name="xp", bufs=2))
        hspool = ctx.enter_context(tc.tile_pool(name="hs", bufs=3))
        opool = ctx.enter_context(tc.tile_pool(name="osb", bufs=3))
        psA = ctx.enter_context(tc.tile_pool(name="psA", bufs=2, space="PSUM"))
        psB = ctx.enter_context(tc.tile_pool(name="psB", bufs=2, space="PSUM"))
        psO = ctx.enter_context(tc.tile_pool(name="psO", bufs=4, space="PSUM"))

        # ---- bulk weight pack: one large DMA
        wpack_sb = wpool.tile([128, WPACK_F], wdt)
        nc.sync.dma_start(out=wpack_sb, in_=wpack_d.ap())

        def wiT_sl(k, m):  # lhsT tile [128, 128] for MM1
            return wpack_sb[:, WI_OFF + k * P + m * 128 : WI_OFF + k * P + (m + 1) * 128]

        def bmat_sl(k, m):
            return wpack_sb[:, B_OFF + k * H + m * 128 : B_OFF + k * H + (m + 1) * 128]

        def w2_sl(k):
            return wpack_sb[:, W2_OFF + k * O : W2_OFF + (k + 1) * O]

        # ---- x^T chunk prefetch (deep, all chunks queued behind the weights)
        xT_v = xT_d.ap().rearrange("(k p) t -> p k t", p=128)
        x_tiles = []
        for c in range(nch):
            x_sb = xpool.tile([128, KD, CH], wdt, name=f"x_sb{c}", tag="x_sb")
            nc.sync.dma_start(out=x_sb, in_=xT_v[:, :, c * CH : (c + 1) * CH])
            x_tiles.append(x_sb)

        # ---- small constants
        mpack_sb = wpool.tile([128, KP + KH], F32)
        nc.sync.dma_start(out=mpack_sb, in_=mpack_d.ap())
        bicol_sl = [mpack_sb[:, m : m + 1] for m in range(KP)]
        acol_sl = [mpack_sb[:, KP + m : KP + m + 1] for m in range(KH)]
        ones_sb = wpool.tile([1, 128], wdt)
        nc.sync.dma_start(out=ones_sb, in_=ones_d.ap())
        borow_sb = wpool.tile([1, O], wdt)
        nc.sync.dma_start(out=borow_sb, in_=borow_d.ap())
        borep_sb = wpool.tile([128, O], F32)
        nc.sync.dma_start(
            out=borep_sb,
            in_=bass.AP(
                tensor=borowf_d.ap().tensor, offset=0, ap=[[0, 128], [1, O]]
            ),
        )

        out_v = out_d.ap().rearrange("(c s p) o -> c p s o", p=128, s=MO)

        hs_tiles = [None] * nch

        def mm3_block(c):
            o_sb = opool.tile([128, MO, O], F32, name=f"o_sb{c}", tag="o_sb")
            hs_sb = hs_tiles[c]
            for st in range(MO):
                ps3 = psO.tile([128, O], F32, tag="ps3", name=f"ps3_{c}_{st}")
                use_pe_bias = st >= n_dve_bias
                for k in range(KH):
                    nc.tensor.matmul(
                        ps3[:, :],
                        hs_sb[:, k, ts(st, 128)],
                        w2_sl(k),
                        start=(k == 0),
                        stop=(k == KH - 1 and not use_pe_bias),
                    )
                if use_pe_bias:
                    nc.tensor.matmul(
                        ps3[:, :], ones_sb[:, :], borow_sb[:, :], start=False, stop=True
                    )
                    nc.scalar.copy(o_sb[:, st, :], ps3[:, :])
                else:
                    nc.vector.tensor_add(o_sb[:, st, :], ps3[:, :], borep_sb[:, :])
            # store on the SWDGE (gpsimd) ring so stores don't queue behind loads
            nc.gpsimd.dma_start(out=out_v[c], in_=o_sb)

        for c in range(nch):
            x_sb = x_tiles[c]

            # ---- MM1 + relu/bias -> xp_sb
            xp_sb = xppool.tile([128, KP, CH], wdt, name=f"xp_sb{c}", tag="xp_sb")
            for m in range(KP):
                ps1 = psA.tile([128, CH], F32, tag="ps1", name=f"ps1_{c}_{m}")
                for k in range(KD):
                    nc.tensor.matmul(
                        ps1[:, :],
                        wiT_sl(k, m),
                        x_sb[:, k, :],
                        start=(k == 0),
                        stop=(k == KD - 1),
                    )
                nc.scalar.activation(
                    out=xp_sb[:, m, :],
                    in_=ps1[:, :],
                    func=mybir.ActivationFunctionType.Relu,
                    bias=bicol_sl[m],
                    scale=1.0,
                )

            # ---- MM2 + scan -> hs_sb
            hs_sb = hspool.tile([128, KH, CH], wdt, name=f"hs_sb{c}", tag="hs_sb")
            for m in range(KH):
                ps2 = psB.tile([128, CH], F32, tag="ps2", name=f"ps2_{c}_{m}")
                for k in range(KP):
                    nc.tensor.matmul(
                        ps2[:, :],
                        bmat_sl(k, m),
                        xp_sb[:, k, :],
                        start=(k == 0),
                        stop=(k == KP - 1),
                    )
                init = (
                    0.0 if c == 0 else hs_tiles[c - 1][:, m, CH - 1 : CH]
                )
                nc.vector.tensor_tensor_scan(
                    out=hs_sb[:, m, :],
                    data0=_bcast_free(acol_sl[m], CH),
                    data1=ps2[:, :],
                    initial=init,
                    op0=mybir.AluOpType.mult,
                    op1=mybir.AluOpType.add,
                )
            hs_tiles[c] = hs_sb

            # ---- deferred MM3 of the previous chunk (keeps PE off the scan's
            # critical path)
            if c > 0:
                mm3_block(c - 1)
        mm3_block(nch - 1)

    nc.finalize()
    _NC_CACHE[key] = nc
    return nc


def _prep_shared(a, B, C, Wi, bi, Wo, bo):
    w2 = (C.astype(np.float64) @ Wo.astype(np.float64).T).astype(np.float32)

    def pack128(w, kt):  # [kt*128, F] -> [128, kt*F]
        return np.transpose(w.reshape(kt, 128, -1), (1, 0, 2)).reshape(128, -1)

    wpack = np.concatenate(
        [
            pack128(np.ascontiguousarray(Wi.T), KD),
            pack128(B, KP),
            pack128(w2, KH),
        ],
        axis=1,
    )
    assert wpack.shape == (128, WPACK_F)
    mpack = np.concatenate(
        [bi.reshape(KP, 128).T, a.reshape(KH, 128).T], axis=1
    ).astype(np.float32)
    shared = {
        "wpack": np.ascontiguousarray(wpack),
        "mpack": np.ascontiguousarray(mpack),
        "ones": np.ones((1, 128), dtype=np.float32),
        "borow": np.ascontiguousarray(bo[None, :]),
        "borowf": np.ascontiguousarray(bo[None, :]),
    }
    return shared


def kernel(x_seq, a, B, C, Wi, bi, Wo, bo, _collect=None):
    nc = build_nc()
    shared = _prep_shared(a, B, C, Wi, bi, Wo, bo)
    in_maps = []
    for b in range(BSZ):
        m = dict(shared)
        m["xT"] = np.ascontiguousarray(x_seq[b].T)
        in_maps.append(m)
    kwargs = {}
    if _collect is not None:
        kwargs = {k: v for k, v in _collect.items() if k != "res"}
    res = run_bass_kernel_spmd(nc, in_maps, core_ids=list(range(BSZ)), **kwargs)
    if _collect is not None:
        _collect["res"] = res
    out = np.stack([res.results[b]["out"] for b in range(BSZ)], axis=0)
    return out


# revision 6
# speedup vs baseline: 1.2460x; 1.2460x over previous
"""Trainium2 Bass kernel for DiscreteTimeS4.

Reference computation (per batch element b):
    x_proj = relu(x @ Wi^T + bi)          [T, P]
    u      = x_proj @ B                   [T, H]
    h_t    = a * h_{t-1} + u_t            (diagonal linear scan over T)
    y      = hs @ C                       [T, P]
    out    = y @ Wo^T + bo                [T, O]

Sharding: data-parallel over the batch axis — core b handles x_seq[b].
Weights replicated. No cross-device communication.

Device strategy (per core, all matmuls in fp32r = full-rate PE):
  - host pre-transposes x to x^T [D, T] so the contraction dim (D) sits on
    SBUF partitions; host also fuses W2 = C @ Wo^T so stages 4+5 collapse
    into one matmul; all [128, *] weights are packed into ONE DRAM tensor so
    startup is a single large DMA.
  - pipeline over time chunks of 512 (PE software-pipelined: chunk c's last
    matmul stage runs while chunk c+1's first stages execute, so the PE
    never waits on the scan):
      MM1: XP^T[p,t] = Wi @ x^T          (lhsT = Wi^T, K=512)
      ACT: relu(psum + bi) -> SBUF (fp32r)
      MM2: U^T[h,t] = B^T @ XP^T         (lhsT = B, K=256)
      DVE: tensor_tensor_scan: h = a*h + u along t (fp32 state, carry
           chained across chunks via initial=prev[:, -1:], decay broadcast
           via a zero-stride AP)
      MM3: OUT[t,o] = (HS^T)^T @ W2 + bo (lhsT = HS^T tile -> natural [t,o]
           output layout; bo folded in as a K=1 matmul of ones^T @ bo_row
           for half the tiles, DVE tensor_add for the rest)
      copy psum -> SBUF (ACT / DVE split), DMA out on the GPSIMD (SWDGE)
      ring so stores never queue behind loads.
"""

import numpy as np

try:
    import concourse.bass as bass
except ImportError:  # pragma: no cover
    import sys

    sys.path.insert(0, "/opt/trn_rl_repo")
    import concourse.bass as bass

from contextlib import ExitStack

import concourse.mybir as mybir
import concourse.tile as tile
from concourse import bacc
from concourse.bass import ts
from concourse.bass_utils import run_bass_kernel_spmd

BSZ, T, D, P, H, O = 8, 4096, 512, 256, 256, 512
CH = 512  # time-chunk (free dim of MM1/MM2, PSUM bank = 512 fp32)
NCH = T // CH
F32 = mybir.dt.float32
F32R = mybir.dt.float32r

# per chunk: how many of the 4 output t-subtiles take the DVE bias-add path
# (remaining go through a K=1 bias matmul on PE + plain ACT copy)
N_DVE_BIAS = 2

KD = D // 128  # 4 k-tiles for MM1
KP = P // 128  # 2
KH = H // 128  # 2
MO = O // 128  # 4 out t-subtiles per chunk

# packed weight layout (free-dim offsets, in fp32 elements)
WI_OFF = 0  # [128, KD*P]
B_OFF = WI_OFF + KD * P  # [128, KP*H]
W2_OFF = B_OFF + KP * H  # [128, KH*O]
WPACK_F = W2_OFF + KH * O

_NC_CACHE = {}


def _bcast_free(ap, n):
    """Broadcast a [128, 1] AP along the free dim via zero stride."""
    return bass.AP(tensor=ap.tensor, offset=ap.offset, ap=[list(ap.ap[0]), [0, n]])


def build_nc(mm_dt=F32R, n_dve_bias=N_DVE_BIAS, nch=NCH):
    key = (mm_dt, n_dve_bias, nch)
    if key in _NC_CACHE:
        return _NC_CACHE[key]
    wdt = mm_dt
    nc = bacc.Bacc("TRN2", target_bir_lowering=False, debug=False)

    xT_d = nc.dram_tensor("xT", [D, T], wdt, kind="ExternalInput")
    wpack_d = nc.dram_tensor("wpack", [128, WPACK_F], wdt, kind="ExternalInput")
    mpack_d = nc.dram_tensor("mpack", [128, KP + KH], F32, kind="ExternalInput")
    ones_d = nc.dram_tensor("ones", [1, 128], wdt, kind="ExternalInput")
    borow_d = nc.dram_tensor("borow", [1, O], wdt, kind="ExternalInput")
    borowf_d = nc.dram_tensor("borowf", [1, O], F32, kind="ExternalInput")
    out_d = nc.dram_tensor("out", [T, O], F32, kind="ExternalOutput")

    with tile.TileContext(nc) as tc, ExitStack() as ctx:
        wpool = ctx.enter_context(tc.tile_pool(name="weights", bufs=1))
        xpool = ctx.enter_context(tc.tile_pool(name="x", bufs=NCH))
        xppool = ctx.enter_context(tc.tile_pool(name="xp", bufs=2))
        hspool = ctx.enter_context(tc.tile_pool(name="hs", bufs=3))
        opool = ctx.enter_context(tc.tile_pool(name="osb", bufs=3))
        psA = ctx.enter_context(tc.tile_pool(name="psA", bufs=2, space="PSUM"))
        psB = ctx.enter_context(tc.tile_pool(name="psB", bufs=2, space="PSUM"))
        psO = ctx.enter_context(tc.tile_pool(name="psO", bufs=4, space="PSUM"))

        # ---- bulk weight pack: one large DMA
        wpack_sb = wpool.tile([128, WPACK_F], wdt)
        nc.sync.dma_start(out=wpack_sb, in_=wpack_d.ap())

        def wiT_sl(k, m):  # lhsT tile [128, 128] for MM1
            return wpack_sb[:, WI_OFF + k * P + m * 128 : WI_OFF + k * P + (m + 1) * 128]

        def bmat_sl(k, m):
            return wpack_sb[:, B_OFF + k * H + m * 128 : B_OFF + k * H + (m + 1) * 128]

        def w2_sl(k):
            return wpack_sb[:, W2_OFF + k * O : W2_OFF + (k + 1) * O]

        # ---- small constants
        mpack_sb = wpool.tile([128, KP + KH], F32)
        nc.sync.dma_start(out=mpack_sb, in_=mpack_d.ap())
        bicol_sl = [mpack_sb[:, m : m + 1] for m in range(KP)]
        acol_sl = [mpack_sb[:, KP + m : KP + m + 1] for m in range(KH)]
        ones_sb = wpool.tile([1, 128], wdt)
        nc.sync.dma_start(out=ones_sb, in_=ones_d.ap())
        borow_sb = wpool.tile([1, O], wdt)
        nc.sync.dma_start(out=borow_sb, in_=borow_d.ap())
        borep_sb = wpool.tile([128, O], F32)
        nc.sync.dma_start(
            out=borep_sb,
            in_=bass.AP(
                tensor=borowf_d.ap().tensor, offset=0, ap=[[0, 128], [1, O]]
            ),
        )

        # ---- x^T chunk prefetch (deep, all chunks queued behind the weights)
        xT_v = xT_d.ap().rearrange("(k p) t -> p k t", p=128)
        x_tiles = []
        for c in range(nch):
            x_sb = xpool.tile([128, KD, CH], wdt, name=f"x_sb{c}", tag="x_sb")
            nc.sync.dma_start(out=x_sb, in_=xT_v[:, :, c * CH : (c + 1) * CH])
            x_tiles.append(x_sb)

        out_v = out_d.ap().rearrange("(c s p) o -> c p s o", p=128, s=MO)

        hs_tiles = [None] * nch

        def mm3_block(c):
            o_sb = opool.tile([128, MO, O], F32, name=f"o_sb{c}", tag="o_sb")
            hs_sb = hs_tiles[c]
            for st in range(MO):
                ps3 = psO.tile([128, O], F32, tag="ps3", name=f"ps3_{c}_{st}")
                use_pe_bias = st >= n_dve_bias
                for k in range(KH):
                    nc.tensor.matmul(
                        ps3[:, :],
                        hs_sb[:, k, ts(st, 128)],
                        w2_sl(k),
                        start=(k == 0),
                        stop=(k == KH - 1 and not use_pe_bias),
                    )
                if use_pe_bias:
                    nc.tensor.matmul(
                        ps3[:, :], ones_sb[:, :], borow_sb[:, :], start=False, stop=True
                    )
                    nc.scalar.copy(o_sb[:, st, :], ps3[:, :])
                else:
                    nc.vector.tensor_add(o_sb[:, st, :], ps3[:, :], borep_sb[:, :])
            # store on the SWDGE (gpsimd) ring so stores don't queue behind loads
            nc.gpsimd.dma_start(out=out_v[c], in_=o_sb)

        for c in range(nch):
            x_sb = x_tiles[c]

            # ---- MM1 + relu/bias -> xp_sb
            xp_sb = xppool.tile([128, KP, CH], wdt, name=f"xp_sb{c}", tag="xp_sb")
            for m in range(KP):
                ps1 = psA.tile([128, CH], F32, tag="ps1", name=f"ps1_{c}_{m}")
                for k in range(KD):
                    nc.tensor.matmul(
                        ps1[:, :],
                        wiT_sl(k, m),
                        x_sb[:, k, :],
                        start=(k == 0),
                        stop=(k == KD - 1),
                    )
                nc.scalar.activation(
                    out=xp_sb[:, m, :],
                    in_=ps1[:, :],
                    func=mybir.ActivationFunctionType.Relu,
                    bias=bicol_sl[m],
                    scale=1.0,
                )

            # ---- MM2 + scan -> hs_sb
            hs_sb = hspool.tile([128, KH, CH], wdt, name=f"hs_sb{c}", tag="hs_sb")
            for m in range(KH):
                ps2 = psB.tile([128, CH], F32, tag="ps2", name=f"ps2_{c}_{m}")
                for k in range(KP):
                    nc.tensor.matmul(
                        ps2[:, :],
                        bmat_sl(k, m),
                        xp_sb[:, k, :],
                        start=(k == 0),
                        stop=(k == KP - 1),
                    )
                init = (
                    0.0 if c == 0 else hs_tiles[c - 1][:, m, CH - 1 : CH]
                )
                nc.vector.tensor_tensor_scan(
                    out=hs_sb[:, m, :],
                    data0=_bcast_free(acol_sl[m], CH),
                    data1=ps2[:, :],
                    initial=init,
                    op0=mybir.AluOpType.mult,
                    op1=mybir.AluOpType.add,
                )
            hs_tiles[c] = hs_sb

            # ---- deferred MM3 of the previous chunk (keeps PE off the scan's
            # critical path)
            if c > 0:
                mm3_block(c - 1)
        mm3_block(nch - 1)

    nc.finalize()
    _NC_CACHE[key] = nc
    return nc


def _prep_shared(a, B, C, Wi, bi, Wo, bo):
    w2 = (C.astype(np.float64) @ Wo.astype(np.float64).T).astype(np.float32)

    def pack128(w, kt):  # [kt*128, F] -> [128, kt*F]
        return np.transpose(w.reshape(kt, 128, -1), (1, 0, 2)).reshape(128, -1)

    wpack = np.concatenate(
        [
            pack128(np.ascontiguousarray(Wi.T), KD),
            pack128(B, KP),
            pack128(w2, KH),
        ],
        axis=1,
    )
    assert wpack.shape == (128, WPACK_F)
    mpack = np.concatenate(
        [bi.reshape(KP, 128).T, a.reshape(KH, 128).T], axis=1
    ).astype(np.float32)
    shared = {
        "wpack": np.ascontiguousarray(wpack),
        "mpack": np.ascontiguousarray(mpack),
        "ones": np.ones((1, 128), dtype=np.float32),
        "borow": np.ascontiguousarray(bo[None, :]),
        "borowf": np.ascontiguousarray(bo[None, :]),
    }
    return shared


def kernel(x_seq, a, B, C, Wi, bi, Wo, bo, _collect=None):
    nc = build_nc()
    shared = _prep_shared(a, B, C, Wi, bi, Wo, bo)
    in_maps = []
    for b in range(BSZ):
        m = dict(shared)
        m["xT"] = np.ascontiguousarray(x_seq[b].T)
        in_maps.append(m)
    kwargs = {}
    if _collect is not None:
        kwargs = {k: v for k, v in _collect.items() if k != "res"}
    res = run_bass_kernel_spmd(nc, in_maps, core_ids=list(range(BSZ)), **kwargs)
    if _collect is not None:
        _collect["res"] = res
    out = np.stack([res.results[b]["out"] for b in range(BSZ)], axis=0)
    return out


# revision 8
# speedup vs baseline: 1.3966x; 1.1208x over previous
"""Trainium2 Bass kernel for DiscreteTimeS4.

Reference computation (per batch element b):
    x_proj = relu(x @ Wi^T + bi)          [T, P]
    u      = x_proj @ B                   [T, H]
    h_t    = a * h_{t-1} + u_t            (diagonal linear scan over T)
    y      = hs @ C                       [T, P]
    out    = y @ Wo^T + bo                [T, O]

Sharding: data-parallel over the batch axis — core b handles x_seq[b].
Weights replicated. No cross-device communication.

Device strategy (per core, all matmuls in fp32r = full-rate PE):
  - host pre-transposes x to x^T [D, T] so the contraction dim (D) sits on
    SBUF partitions; host also fuses W2 = C @ Wo^T so stages 4+5 collapse
    into one matmul; all [128, *] weights are packed into ONE DRAM tensor so
    startup is a single large DMA.
  - pipeline over time chunks of 512 (PE software-pipelined: chunk c's last
    matmul stage runs while chunk c+1's first stages execute, so the PE
    never waits on the scan):
      MM1: XP^T[p,t] = Wi @ x^T          (lhsT = Wi^T, K=512)
      ACT: relu(psum + bi) -> SBUF (fp32r)
      MM2: U^T[h,t] = B^T @ XP^T         (lhsT = B, K=256)
      DVE: tensor_tensor_scan: h = a*h + u along t (fp32 state, carry
           chained across chunks via initial=prev[:, -1:], decay broadcast
           via a zero-stride AP)
      MM3: OUT[t,o] = (HS^T)^T @ W2 + bo (lhsT = HS^T tile -> natural [t,o]
           output layout; bo folded in as a K=1 matmul of ones^T @ bo_row
           for half the tiles, DVE tensor_add for the rest)
      copy psum -> SBUF (ACT / DVE split), DMA out on the GPSIMD (SWDGE)
      ring so stores never queue behind loads.
"""

import numpy as np

try:
    import concourse.bass as bass
except ImportError:  # pragma: no cover
    import sys

    sys.path.insert(0, "/opt/trn_rl_repo")
    import concourse.bass as bass

from contextlib import ExitStack

import concourse.mybir as mybir
import concourse.tile as tile
from concourse import bacc
from concourse.bass import ts
from concourse.bass_utils import run_bass_kernel_spmd

BSZ, T, D, P, H, O = 8, 4096, 512, 256, 256, 512
CH = 512  # time-chunk (free dim of MM1/MM2, PSUM bank = 512 fp32)
NCH = T // CH
F32 = mybir.dt.float32
F32R = mybir.dt.float32r

# per chunk: how many of the 4 output t-subtiles take the DVE bias-add path
# (remaining go through a K=1 bias matmul on PE + plain ACT copy)
N_DVE_BIAS = 2

KD = D // 128  # 4 k-tiles for MM1
KP = P // 128  # 2
KH = H // 128  # 2
MO = O // 128  # 4 out t-subtiles per chunk

# packed weight layout (free-dim offsets, in fp32 elements)
WI_OFF = 0  # [128, KD*P]
B_OFF = WI_OFF + KD * P  # [128, KP*H]
W2_OFF = B_OFF + KP * H  # [128, KH*O]
WPACK_F = W2_OFF + KH * O

_NC_CACHE = {}


def _bcast_free(ap, n):
    """Broadcast a [128, 1] AP along the free dim via zero stride."""
    return bass.AP(tensor=ap.tensor, offset=ap.offset, ap=[list(ap.ap[0]), [0, n]])


def build_nc(mm_dt=F32R, n_dve_bias=N_DVE_BIAS, nch=NCH):
    key = (mm_dt, n_dve_bias, nch)
    if key in _NC_CACHE:
        return _NC_CACHE[key]
    wdt = mm_dt
    nc = bacc.Bacc("TRN2", target_bir_lowering=False, debug=False)

    xT_d = nc.dram_tensor("xT", [D, T], wdt, kind="ExternalInput")
    wpack_d = nc.dram_tensor("wpack", [128, WPACK_F], wdt, kind="ExternalInput")
    mpack_d = nc.dram_tensor("mpack", [128, KP + KH], F32, kind="ExternalInput")
    borowf_d = nc.dram_tensor("borowf", [1, O], F32, kind="ExternalInput")
    out_d = nc.dram_tensor("out", [T, O], F32, kind="ExternalOutput")

    with tile.TileContext(nc) as tc, ExitStack() as ctx:
        wpool = ctx.enter_context(tc.tile_pool(name="weights", bufs=1))
        xpool = ctx.enter_context(tc.tile_pool(name="x", bufs=NCH))
        xppool = ctx.enter_context(tc.tile_pool(name="xp", bufs=2))
        hspool = ctx.enter_context(tc.tile_pool(name="hs", bufs=3))
        opool = ctx.enter_context(tc.tile_pool(name="osb", bufs=3))
        psA = ctx.enter_context(tc.tile_pool(name="psA", bufs=2, space="PSUM"))
        psB = ctx.enter_context(tc.tile_pool(name="psB", bufs=2, space="PSUM"))
        psO = ctx.enter_context(tc.tile_pool(name="psO", bufs=4, space="PSUM"))

        # ---- bulk weight pack: wi+b first (MM1/MM2 can start), then x0,
        # then w2 and the small constants, then the deep x prefetch
        wpack_sb = wpool.tile([128, WPACK_F], wdt)
        nc.sync.dma_start(
            out=wpack_sb[:, :W2_OFF], in_=wpack_d.ap()[:, :W2_OFF]
        )

        def wiT_sl(k, m):  # lhsT tile [128, 128] for MM1
            return wpack_sb[:, WI_OFF + k * P + m * 128 : WI_OFF + k * P + (m + 1) * 128]

        def bmat_sl(k, m):
            return wpack_sb[:, B_OFF + k * H + m * 128 : B_OFF + k * H + (m + 1) * 128]

        def w2_sl(k):
            return wpack_sb[:, W2_OFF + k * O : W2_OFF + (k + 1) * O]

        xT_v = xT_d.ap().rearrange("(k p) t -> p k t", p=128)
        x_tiles = []
        x0_sb = xpool.tile([128, KD, CH], wdt, name="x_sb0", tag="x_sb")
        nc.sync.dma_start(out=x0_sb[:, :2, :], in_=xT_v[:, :2, 0:CH])
        nc.sync.dma_start(out=x0_sb[:, 2:, :], in_=xT_v[:, 2:, 0:CH])
        x_tiles.append(x0_sb)
        nc.sync.dma_start(
            out=wpack_sb[:, W2_OFF:], in_=wpack_d.ap()[:, W2_OFF:]
        )

        # ---- small constants
        mpack_sb = wpool.tile([128, KP + KH], F32)
        nc.sync.dma_start(out=mpack_sb, in_=mpack_d.ap())
        bicol_sl = [mpack_sb[:, m : m + 1] for m in range(KP)]
        acol_sl = [mpack_sb[:, KP + m : KP + m + 1] for m in range(KH)]
        borep_sb = wpool.tile([128, O], F32)
        nc.sync.dma_start(
            out=borep_sb,
            in_=bass.AP(
                tensor=borowf_d.ap().tensor, offset=0, ap=[[0, 128], [1, O]]
            ),
        )

        # ---- deep x prefetch for the remaining chunks
        for c in range(1, nch):
            x_sb = xpool.tile([128, KD, CH], wdt, name=f"x_sb{c}", tag="x_sb")
            nc.sync.dma_start(out=x_sb, in_=xT_v[:, :, c * CH : (c + 1) * CH])
            x_tiles.append(x_sb)

        out_v = out_d.ap().rearrange("(c s p) o -> c p s o", p=128, s=MO)

        hs_tiles = [None] * nch

        def mm3_block(c, split_store=False):
            o_sb = opool.tile([128, MO, O], F32, name=f"o_sb{c}", tag="o_sb")
            hs_sb = hs_tiles[c]
            for st in range(MO):
                ps3 = psO.tile([128, O], F32, tag="ps3", name=f"ps3_{c}_{st}")
                for k in range(KH):
                    nc.tensor.matmul(
                        ps3[:, :],
                        hs_sb[:, k, ts(st, 128)],
                        w2_sl(k),
                        start=(k == 0),
                        stop=(k == KH - 1),
                    )
                if st >= n_dve_bias:
                    # ACT copies out of PSUM, idle GPSIMD applies the bias
                    nc.scalar.copy(o_sb[:, st, :], ps3[:, :])
                    nc.gpsimd.tensor_add(
                        o_sb[:, st, :], o_sb[:, st, :], borep_sb[:, :]
                    )
                else:
                    nc.vector.tensor_add(o_sb[:, st, :], ps3[:, :], borep_sb[:, :])
                if split_store:
                    nc.gpsimd.dma_start(
                        out=out_v[c][:, st, :], in_=o_sb[:, st, :]
                    )
            if not split_store:
                # store on the SWDGE (gpsimd) ring, off the load ring
                nc.gpsimd.dma_start(out=out_v[c], in_=o_sb)

        for c in range(nch):
            x_sb = x_tiles[c]

            # ---- MM1 + relu/bias -> xp_sb
            xp_sb = xppool.tile([128, KP, CH], wdt, name=f"xp_sb{c}", tag="xp_sb")
            for m in range(KP):
                ps1 = psA.tile([128, CH], F32, tag="ps1", name=f"ps1_{c}_{m}")
                for k in range(KD):
                    nc.tensor.matmul(
                        ps1[:, :],
                        wiT_sl(k, m),
                        x_sb[:, k, :],
                        start=(k == 0),
                        stop=(k == KD - 1),
                    )
                nc.scalar.activation(
                    out=xp_sb[:, m, :],
                    in_=ps1[:, :],
                    func=mybir.ActivationFunctionType.Relu,
                    bias=bicol_sl[m],
                    scale=1.0,
                )

            # ---- MM2 + scan -> hs_sb
            hs_sb = hspool.tile([128, KH, CH], wdt, name=f"hs_sb{c}", tag="hs_sb")
            for m in range(KH):
                ps2 = psB.tile([128, CH], F32, tag="ps2", name=f"ps2_{c}_{m}")
                for k in range(KP):
                    nc.tensor.matmul(
                        ps2[:, :],
                        bmat_sl(k, m),
                        xp_sb[:, k, :],
                        start=(k == 0),
                        stop=(k == KP - 1),
                    )
                init = (
                    0.0 if c == 0 else hs_tiles[c - 1][:, m, CH - 1 : CH]
                )
                nc.vector.tensor_tensor_scan(
                    out=hs_sb[:, m, :],
                    data0=_bcast_free(acol_sl[m], CH),
                    data1=ps2[:, :],
                    initial=init,
                    op0=mybir.AluOpType.mult,
                    op1=mybir.AluOpType.add,
                )
            hs_tiles[c] = hs_sb

            # ---- deferred MM3 of the previous chunk (keeps PE off the scan's
            # critical path)
            if c > 0:
                mm3_block(c - 1)
        mm3_block(nch - 1, split_store=True)

    nc.finalize()
    _NC_CACHE[key] = nc
    return nc


def _prep_shared(a, B, C, Wi, bi, Wo, bo):
    w2 = (C.astype(np.float64) @ Wo.astype(np.float64).T).astype(np.float32)

    def pack128(w, kt):  # [kt*128, F] -> [128, kt*F]
        return np.transpose(w.reshape(kt, 128, -1), (1, 0, 2)).reshape(128, -1)

    wpack = np.concatenate(
        [
            pack128(np.ascontiguousarray(Wi.T), KD),
            pack128(B, KP),
            pack128(w2, KH),
        ],
        axis=1,
    )
    assert wpack.shape == (128, WPACK_F)
    mpack = np.concatenate(
        [bi.reshape(KP, 128).T, a.reshape(KH, 128).T], axis=1
    ).astype(np.float32)
    shared = {
        "wpack": np.ascontiguousarray(wpack),
        "mpack": np.ascontiguousarray(mpack),
        "borowf": np.ascontiguousarray(bo[None, :]),
    }
    return shared


def kernel(x_seq, a, B, C, Wi, bi, Wo, bo, _collect=None):
    nc = build_nc()
    shared = _prep_shared(a, B, C, Wi, bi, Wo, bo)
    in_maps = []
    for b in range(BSZ):
        m = dict(shared)
        m["xT"] = np.ascontiguousarray(x_seq[b].T)
        in_maps.append(m)
    kwargs = {}
    if _collect is not None:
        kwargs = {k: v for k, v in _collect.items() if k != "res"}
    res = run_bass_kernel_spmd(nc, in_maps, core_ids=list(range(BSZ)), **kwargs)
    if _collect is not None:
        _collect["res"] = res
    out = np.stack([res.results[b]["out"] for b in range(BSZ)], axis=0)
    return out
